# revision 28
# baseline (speedup 1.0000x reference)
"""Trainium2 Bass kernel for a dense transformer decoder block.

reference: x:(2,2048,1024) f32; LN1 -> causal MHA (16 heads, d=64) -> +res;
LN2 -> MLP (d_ff=4096, exact gelu) -> +res.

Sharding: stride-4 interleaved token-parallel.  Core i (batch b=i//4, rank
k=i%4) owns tokens x[b, k::4] (512 tokens).  With this assignment, local
query block t (128 queries, global positions 4*(128t+qq)+k) attends exactly
the gathered key blocks (r, jj<=t) -- the causal skip pattern is identical
on every core (40 of 64 key-block visits), so one uniform SPMD program is
load-balanced AND skips ~38% of attention work.

K/V are exchanged in fp8e4 wire format with FOUR per-head-group AllGathers
fired as soon as each group's K/V projections finish.  All kvout->SBUF
fetches are emitted up-front on the sync queue in group (= arrival)
order so nothing else ever queues behind an ungathered group.

Attention per (g,o): QK (fp8 lhsT) -> exp (ACT, -25 bias keeps masked
garbage negligible) -> diagonal-only mask -> PV with ones-column
denominators (ones = WSCALE so den cancels V's weight scale).  Softmax
normalization: exact DVE reciprocal per head (approx variants are wrong
for the tiny -25-biased denominators), ones-matmul row broadcast into a
recycled qk-pool PSUM tile, TT mults; PV PSUM is double-buffered so
head-pair tails overlap the next head-pair's QK/exp/PV.  A ~8us
garbage-matmul warmup at t=0 ramps the PE DVFS pstate early.

Matmul compute is bf16 (fp8 lhsT for attention K/V), fp32 PSUM.
"""

import numpy as np

B, S, H = 2, 2048, 1024
NCORES = 8
TOK = 512            # tokens per core
HEADS, D = 16, 64
DFF = 4096
EPS = 1e-5

KVK = 128 * 2 * 512           # K wire region [p, o, t]
KVV = 4 * 128 * 260           # V wire region [tt, p, c] (65 cols/head incl ones)
KVE = KVK + KVV

# fp8e4m3 min normal is 2^-6; raw weights (sigma=0.02) would be subnormal.
# Store weights*WSCALE in fp8 and undo the factor downstream (exp scale,
# ones-column value, gelu input scale, out-proj residual add).
WSCALE = 32.0

_PROG = None         # cached compiled program


def _build_program():
    import concourse.mybir as mybir
    import concourse.tile as tile
    from concourse import bacc
    from concourse.masks import make_identity

    dt = mybir.dt
    f32, bf16, f8 = dt.float32, dt.bfloat16, dt.float8e4
    AF = mybir.ActivationFunctionType
    ALU = mybir.AluOpType
    DR = mybir.MatmulPerfMode.DoubleRow

    nc = bacc.Bacc("TRN2", target_bir_lowering=False, debug=False,
                   num_devices=NCORES)

    # ---- kernel I/O (weights pre-rearranged on host for contiguous DMA) ----
    xc_d = nc.dram_tensor("xc", [TOK, H], f32, kind="ExternalInput")
    # wqk[p, ot, hc, o] = (ln1w*wqkv*WS)[128*hc+p, 128*ot+o]; ot 0-7 q, 8-15 k
    wqk_d = nc.dram_tensor("wqk", [128, 16, 8, 128], f8, kind="ExternalInput")
    # wv[p, hc, c] = (ln1w*wqkv*WS)[128*hc+p, 2048+c]
    wv_d = nc.dram_tensor("wv", [128, 8, 1024], f8, kind="ExternalInput")
    # wout[64*hp+d, hh, no, o] = (w_out*WS)[64*(2*hh+hp)+d, 512*no+o]
    wout_d = nc.dram_tensor("wout", [128, 8, 2, 512], f8,
                            kind="ExternalInput")
    # wfc1[p, f, hc, o] = (ln2w*w_fc1*WS)[128*hc+p, 128*f+o]
    wfc1_d = nc.dram_tensor("wfc1", [128, 32, 8, 128], f8,
                            kind="ExternalInput")
    # wfc2[p, f, c] = w_fc2[128*f+p, c]
    wfc2_d = nc.dram_tensor("wfc2", [128, 32, 1024], bf16,
                            kind="ExternalInput")
    # mk[p, r, hp, qq] = 1 if 4p + r <= 4qq + k else 0   (diagonal block mask)
    mk_d = nc.dram_tensor("mk", [128, 4, 2, 128], bf16, kind="ExternalInput")
    out_d = nc.dram_tensor("out", [TOK, H], f32, kind="ExternalOutput")

    GROUPS = [[0, 1, 2, 3], [4, 5, 6, 7]]

    # per-head-group K/V exchange buffers (fp8 wire)
    kvin = [nc.dram_tensor(f"kvin{g}", [KVE], f8, kind="Internal")
            for g in range(4)]
    kvout = [nc.dram_tensor(f"kvout{g}", [4 * KVE], f8, kind="Internal")
             for g in range(4)]

    def kvin_k(g):
        return (kvin[g].ap()[0:KVK]
                .rearrange("(p o t) -> p o t", p=128, o=2, t=512))

    def kvin_v(g):
        return (kvin[g].ap()[KVK:KVE]
                .rearrange("(tt p c) -> p tt c", tt=4, p=128, c=260))

    def kvout_k(g):
        return (kvout[g].ap().rearrange("(r x) -> r x", r=4)[:, 0:KVK]
                .rearrange("r (p o t) -> p r o t", p=128, o=2, t=512))

    def kvout_v(g):
        return (kvout[g].ap().rearrange("(r x) -> r x", r=4)[:, KVK:KVE]
                .rearrange("r (tt p c) -> p r tt c", tt=4, p=128, c=260))

    with tile.TileContext(nc) as tc:
        with tc.tile_pool(name="persist", bufs=1) as pp:
            # ---------- persistent SBUF ----------
            x_sb = pp.tile([128, 4, H], f32, tag="x_sb")      # x[128*tt+p, h]
            xlt = pp.tile([128, 8, TOK], f8, tag="xlt")       # ln(x)^T
            qt = pp.tile([128, 8, TOK], bf16, tag="qt")       # q^T (x WSCALE)
            at2 = pp.tile([128, 8, TOK], f8, tag="at2")       # attn out, 2-head packed
            h1g = pp.tile([128, 32, TOK], bf16, tag="h1g")    # gelu(fc1)
            mk_sb = pp.tile([128, 4, 2, 128], bf16, tag="mk")
            wv_sb = pp.tile([128, 8, 1024], f8, tag="wv_sb")
            wout_sb = pp.tile([128, 8, 2, TOK], f8, tag="wout_sb")
            onesb = pp.tile([128, 64], bf16, tag="onesb")
            ident = pp.tile([128, 128], bf16, tag="ident")
            eps_sb = pp.tile([128, 1], f32, tag="eps_sb")
            nb25 = pp.tile([128, 1], f32, tag="nb25")

            # ---------- constants / initial DMAs ----------
            nc.sync.dma_start(mk_sb[:], mk_d.ap())
            x_view = xc_d.ap().rearrange("(tt p) h -> p tt h", p=128)
            for tt in range(4):
                nc.sync.dma_start(x_sb[:, tt, :], x_view[:, tt, :])
            nc.sync.dma_start(wv_sb[:], wv_d.ap())
            nc.gpsimd.memset(eps_sb[:], EPS)
            nc.gpsimd.memset(nb25[:], -25.0)
            nc.gpsimd.memset(onesb[:], 1.0)
            make_identity(nc, ident[:])

            # ---------- LayerNorm + transpose helper ----------
            def layernorm_t(xlt_dst, ln_pool, tp_ps_pool):
                # batched so Ln and Exp each load their ACT table once
                rstd = ln_pool.tile([128, 4], f32, tag="ln_rstd")
                nmr = ln_pool.tile([128, 4], f32, tag="ln_nmr")
                aggs = []
                for tt in range(4):
                    xrow = x_sb[:, tt, :]
                    st = ln_pool.tile([128, 2, 6], f32, tag="ln_st",
                                      name=f"ln_st{tt}")
                    nc.vector.bn_stats(st[:, 0, :], xrow[:, 0:512])
                    nc.vector.bn_stats(st[:, 1, :], xrow[:, 512:1024])
                    agg = ln_pool.tile([128, 2], f32, tag="ln_agg",
                                       name=f"ln_agg{tt}")
                    nc.vector.bn_aggr(agg[:], st[:])
                    aggs.append(agg)
                    # rstd = exp(-0.5 * ln(var + eps))
                    nc.scalar.activation(rstd[:, tt:tt + 1], agg[:, 1:2],
                                         AF.Ln, bias=eps_sb[:])
                for tt in range(4):
                    nc.scalar.activation(rstd[:, tt:tt + 1], rstd[:, tt:tt + 1],
                                         AF.Exp, scale=-0.5)
                    nc.vector.tensor_tensor(nmr[:, tt:tt + 1],
                                            aggs[tt][:, 0:1],
                                            rstd[:, tt:tt + 1], ALU.mult)
                    nc.vector.tensor_scalar_mul(nmr[:, tt:tt + 1],
                                                nmr[:, tt:tt + 1], -1.0)
                xlns = []
                for tt in range(4):
                    xln = ln_pool.tile([128, H], bf16, tag="ln_out",
                                       name=f"ln_out{tt}")
                    nc.scalar.activation(xln[:], x_sb[:, tt, :], AF.Identity,
                                         bias=nmr[:, tt:tt + 1],
                                         scale=rstd[:, tt:tt + 1])
                    xlns.append(xln)
                # hc-major: each xlt[:, 2c:2c+2, :] (one matmul c-chunk)
                # completes after 8 transposes, so projections start early;
                # PSUM->SBUF copies alternate vector/scalar to halve the
                # per-engine copy load
                for hc in range(8):
                    for tt in range(4):
                        tp = tp_ps_pool.tile([128, 128], bf16, tag="tp")
                        nc.tensor.transpose(
                            tp[:], xlns[tt][:, 128 * hc:128 * (hc + 1)],
                            ident[:])
                        eng = nc.vector.tensor_copy if (tt % 2 == 0) \
                            else nc.scalar.copy
                        eng(xlt_dst[:, hc, 128 * tt:128 * (tt + 1)], tp[:])

            # PE warmup: ~8us of garbage matmuls while the input DMAs
            # stream, so the DVFS pstate ramps before the projections;
            # results are never read (uninit fp8 NaNs are harmless in PSUM)
            with tc.tile_pool(name="warm_ps", bufs=1, space="PSUM") as wps:
                warm = wps.tile([128, TOK], f32, tag="warm")
                for i in range(16):
                    nc.tensor.matmul(warm[:], xlt[:, 0:2, 0:128],
                                     xlt[:, 0:2, :],
                                     start=True, stop=True, perf_mode=DR)

            with (
                tc.tile_pool(name="ln1", bufs=4) as ln_pool,
                tc.tile_pool(name="tp_ps", bufs=2, space="PSUM") as tp_ps,
            ):
                layernorm_t(xlt, ln_pool, tp_ps)

            # ---------- K + V projections per head group; fire AllGathers ----
            with (
                tc.tile_pool(name="wqk_p", bufs=3) as wqk_p,
                tc.tile_pool(name="mm_ps", bufs=3, space="PSUM") as mm_ps,
                tc.tile_pool(name="kstage", bufs=2) as kstage,
                tc.tile_pool(name="vstage", bufs=2) as vstage,
            ):
                for g in range(4):
                    kb = kstage.tile([128, 2, TOK], f8, tag="kb")
                    for oo in range(2):
                        ot = 8 + 2 * g + oo
                        wq = wqk_p.tile([128, 8, 128], f8, tag="wqk")
                        nc.sync.dma_start(wq[:], wqk_d.ap()[:, ot, :, :])
                        ps = mm_ps.tile([128, TOK], f32, tag="mm")
                        for c in range(4):
                            nc.tensor.matmul(ps[:], wq[:, 2 * c:2 * c + 2, :],
                                             xlt[:, 2 * c:2 * c + 2, :],
                                             start=(c == 0), stop=(c == 3),
                                             perf_mode=DR)
                        nc.scalar.copy(kb[:, oo, :], ps[:])
                    nc.sync.dma_start(kvin_k(g), kb[:])

                    # va[p, tt, hl, 65] = [d x64, ones]; ones = WSCALE so
                    # each head's denominator row (partition 64 of its PV
                    # matmul) comes out pre-multiplied (cancels V's WSCALE)
                    va = vstage.tile([128, 4, 4, 65], f8, tag="va")
                    nc.gpsimd.memset(va[:, :, :, 64:65], WSCALE)
                    for tt in range(4):
                        ps = mm_ps.tile([128, 256], f32, tag="mmv")
                        for c in range(4):
                            nc.tensor.matmul(
                                ps[:],
                                xlt[:, 2 * c:2 * c + 2,
                                    128 * tt:128 * (tt + 1)],
                                wv_sb[:, 2 * c:2 * c + 2,
                                      256 * g:256 * (g + 1)],
                                start=(c == 0), stop=(c == 3), perf_mode=DR)
                        nc.scalar.copy(
                            va[:, tt, :, 0:64],
                            ps[:].rearrange("p (h e) -> p h e", e=64))
                    nc.sync.dma_start(
                        kvin_v(g), va[:].rearrange("p tt h e -> p tt (h e)"))

                    nc.gpsimd.collective_compute(
                        "AllGather", mybir.AluOpType.bypass,
                        replica_groups=GROUPS,
                        ins=[kvin[g].ap()], outs=[kvout[g].ap()])

                # ---------- Q projection (overlaps first AllGathers) ----------
                for ot in range(8):
                    wq = wqk_p.tile([128, 8, 128], f8, tag="wqk")
                    nc.sync.dma_start(wq[:], wqk_d.ap()[:, ot, :, :])
                    ps = mm_ps.tile([128, TOK], f32, tag="mm")
                    for c in range(4):
                        nc.tensor.matmul(ps[:], wq[:, 2 * c:2 * c + 2, :],
                                         xlt[:, 2 * c:2 * c + 2, :],
                                         start=(c == 0), stop=(c == 3),
                                         perf_mode=DR)
                    nc.vector.tensor_copy(qt[:, ot, :], ps[:])

            # prefetch out-proj weights + first MLP weight chunks while
            # collectives run (ungated: must precede gather-gated fetches
            # on the in-order sync queue)
            nc.sync.dma_start(wout_sb[:], wout_d.ap())

            with (
                tc.tile_pool(name="wfc1_p", bufs=2) as wfc1_p,
                tc.tile_pool(name="wfc2_p", bufs=2) as wfc2_p,
            ):
                wfc1_t0 = wfc1_p.tile([128, 4, 8, 128], f8, tag="wfc1")
                nc.sync.dma_start(wfc1_t0[:], wfc1_d.ap()[:, 0:4, :, :])
                wfc2_t0 = wfc2_p.tile([128, 8, TOK], bf16, tag="wfc2")
                nc.sync.dma_start(wfc2_t0[:],
                                  wfc2_d.ap()[:, 0:8, 0:TOK])

                # ---------- attention ----------
                with (
                    tc.tile_pool(name="kt_p", bufs=2) as kt_p,
                    tc.tile_pool(name="vg_p", bufs=2) as vg_p,
                    tc.tile_pool(name="qk_ps", bufs=2, space="PSUM") as qk_ps,
                    tc.tile_pool(name="pv_ps", bufs=2, space="PSUM") as pv_ps,
                    tc.tile_pool(name="exp_p", bufs=3) as exp_p,
                    tc.tile_pool(name="den_p", bufs=2) as den_p,
                    tc.tile_pool(name="rb_p", bufs=2) as rb_p,
                    tc.tile_pool(name="tmp_p", bufs=2) as tmp_p,
                ):
                    # front-load ALL gather-gated fetches in group (=arrival)
                    # order; nothing else shares the queue behind them
                    kts, vgs = [], []
                    for g in range(4):
                        kt = kt_p.tile([128, 4, 2, TOK], f8, tag="kt",
                                       name=f"kt{g}")
                        nc.sync.dma_start(kt[:], kvout_k(g))
                        vg = vg_p.tile([128, 4, 4, 260], f8, tag="vg",
                                       name=f"vg{g}")
                        for r in range(4):
                            nc.sync.dma_start(vg[:, r, :, :],
                                              kvout_v(g)[:, r, :, :])
                        kts.append(kt)
                        vgs.append(vg)

                    for g in range(4):
                        kt, vg = kts[g], vgs[g]
                        for o in range(2):
                            hh = 2 * g + o
                            psO = pv_ps.tile([65, 2, TOK], f32, tag="pv")
                            pend = None      # 1-deep software pipeline for PV
                            for j in range(16):
                                r, jj = j // 4, j % 4
                                N = 512 - 128 * jj
                                ps = qk_ps.tile([128, 2, TOK], f32, tag="qk")
                                for hp in range(2):
                                    nc.tensor.matmul(
                                        ps[:, hp, 0:N],
                                        kt[64 * hp:64 * (hp + 1), r, o,
                                           128 * jj:128 * (jj + 1)],
                                        qt[64 * hp:64 * (hp + 1), hh,
                                           128 * jj:512],
                                        start=True, stop=True)
                                ex = exp_p.tile([128, 2, TOK], bf16, tag="exp")
                                # -25 bias keeps masked-position garbage exps
                                # tiny -- it cancels in softmax; mask-mult
                                # after exp gives exact zeros.
                                # q,k each carry WSCALE -> logits x WSCALE^2
                                nc.scalar.activation(ex[:, :, 0:N],
                                                     ps[:, :, 0:N],
                                                     AF.Exp,
                                                     scale=0.125 / (WSCALE ** 2),
                                                     bias=nb25[:])
                                # only the diagonal sub-block (first 128 query
                                # cols of this tile) straddles causality
                                nc.vector.tensor_tensor(
                                    ex[:, :, 0:128], ex[:, :, 0:128],
                                    mk_sb[:, r, :, :], ALU.mult)
                                if pend is not None:
                                    pend()

                                def make_pv(r=r, jj=jj, N=N, ex=ex, j=j,
                                            psO=psO, vg=vg, o=o):
                                    # both heads at base partition 0, each
                                    # with its own den row at partition 64
                                    # of its hp column range
                                    def one(hp, cols_lo, cols_hi, ex_lo, ex_hi,
                                            st, sp):
                                        hl = 2 * o + hp
                                        lhsT = vg[:, r, jj,
                                                  65 * hl:65 * (hl + 1)]
                                        nc.tensor.matmul(
                                            psO[0:65, hp, cols_lo:cols_hi],
                                            lhsT,
                                            ex[:, hp, ex_lo:ex_hi],
                                            start=st, stop=sp)

                                    def pv():
                                        for hp in range(2):
                                            if r == 3:
                                                one(hp, 128 * jj,
                                                    128 * (jj + 1),
                                                    0, 128, j == 0, True)
                                                if jj < 3:
                                                    one(hp, 128 * (jj + 1),
                                                        512, 128, N,
                                                        j == 0, False)
                                            else:
                                                one(hp, 128 * jj, 512,
                                                    0, N, j == 0, False)
                                    return pv
                                pend = make_pv()
                            pend()

                            # normalize: at2 = psO_d * (1/den); den rows
                            # already carry WSCALE from the va ones column.
                            # One batched approx reciprocal (both heads'
                            # den rows at partition 64), ones-matmul row
                            # broadcast into a recycled qk-pool PSUM tile
                            # (no extra bank), hp1 bounced to partitions
                            # 64-127 via a small GpSimd-queue DMA.
                            dnb = den_p.tile([128, 2, TOK], bf16, tag="dnb")
                            rb = rb_p.tile([128, 2, TOK], bf16, tag="rb")
                            tmp = tmp_p.tile([64, TOK], f8, tag="tmp")
                            br = qk_ps.tile([128, 2, TOK], f32, tag="qk",
                                            name=f"br{g}_{o}")
                            # NOTE: reciprocal_approx_fast is numerically
                            # wrong for the ~1e-11 denominators the -25 exp
                            # bias produces; exact DVE reciprocal required
                            with nc.allow_low_precision(
                                    reason="softmax recip in bf16"):
                                nc.vector.reciprocal(dnb[64:65, 0, :],
                                                     psO[64:65, 0, :])
                                nc.vector.reciprocal(dnb[64:65, 1, :],
                                                     psO[64:65, 1, :])
                            for hp in range(2):
                                nc.tensor.matmul(br[0:64, hp, :],
                                                 onesb[64:65, 0:64],
                                                 dnb[64:65, hp, :],
                                                 start=True, stop=True)
                            nc.vector.tensor_copy(rb[0:64, :, :],
                                                  br[0:64, :, :])
                            nc.vector.tensor_tensor(
                                at2[0:64, hh, :], psO[0:64, 0, :],
                                rb[0:64, 0, :], ALU.mult)
                            nc.vector.tensor_tensor(
                                tmp[:], psO[0:64, 1, :],
                                rb[0:64, 1, :], ALU.mult)
                            nc.sync.dma_start(at2[64:128, hh, :], tmp[:])

                # ---------- out projection + residual (in place on x_sb);
                # tt-major so LN2 stats pipeline behind it ----------
                with tc.tile_pool(name="mm2_ps", bufs=2,
                                  space="PSUM") as mm2_ps:
                    for tt in range(4):
                        for no in range(2):
                            ps = mm2_ps.tile([128, TOK], f32, tag="mm2")
                            for c in range(4):
                                nc.tensor.matmul(
                                    ps[:],
                                    at2[:, 2 * c:2 * c + 2,
                                        128 * tt:128 * (tt + 1)],
                                    wout_sb[:, 2 * c:2 * c + 2, no, :],
                                    start=(c == 0), stop=(c == 3),
                                    perf_mode=DR)
                            # w_out carries WSCALE: undo while adding residual
                            nc.vector.scalar_tensor_tensor(
                                x_sb[:, tt, TOK * no:TOK * (no + 1)], ps[:],
                                1.0 / WSCALE,
                                x_sb[:, tt, TOK * no:TOK * (no + 1)],
                                ALU.mult, ALU.add)

                # ---------- LN2 + transpose (reuses xlt) ----------
                with (
                    tc.tile_pool(name="ln2", bufs=4) as ln2_pool,
                    tc.tile_pool(name="tp2_ps", bufs=2, space="PSUM") as tp2_ps,
                ):
                    layernorm_t(xlt, ln2_pool, tp2_ps)

                # ---------- MLP fc1 + gelu ----------
                with tc.tile_pool(name="fc1_ps", bufs=2,
                                  space="PSUM") as fc1_ps:
                    for ch in range(8):
                        if ch == 0:
                            wt = wfc1_t0
                        else:
                            wt = wfc1_p.tile([128, 4, 8, 128], f8, tag="wfc1")
                            nc.sync.dma_start(
                                wt[:], wfc1_d.ap()[:, 4 * ch:4 * (ch + 1),
                                                   :, :])
                        for fi in range(4):
                            f = 4 * ch + fi
                            ps = fc1_ps.tile([128, TOK], f32, tag="fc1")
                            for c in range(4):
                                nc.tensor.matmul(ps[:],
                                                 wt[:, fi, 2 * c:2 * c + 2, :],
                                                 xlt[:, 2 * c:2 * c + 2, :],
                                                 start=(c == 0), stop=(c == 3),
                                                 perf_mode=DR)
                            # w_fc1 carries WSCALE: gelu(ps/WSCALE)
                            nc.scalar.activation(h1g[:, f, :], ps[:], AF.Gelu,
                                                 scale=1.0 / WSCALE)

                # ---------- MLP fc2 + residual -> out ----------
                with (
                    tc.tile_pool(name="fc2_ps", bufs=4, space="PSUM") as fc2_ps,
                    tc.tile_pool(name="o_p", bufs=2) as o_p,
                ):
                    out_view = out_d.ap().rearrange("(tt p) h -> p tt h",
                                                    p=128)
                    for no in range(2):
                        pss = [fc2_ps.tile([128, TOK], f32, tag="fc2",
                                           name=f"fc2ps_{no}_{tt}")
                               for tt in range(4)]
                        for ch in range(4):
                            if no == 0 and ch == 0:
                                wt = wfc2_t0
                            else:
                                wt = wfc2_p.tile([128, 8, TOK], bf16,
                                                 tag="wfc2")
                                nc.sync.dma_start(
                                    wt[:], wfc2_d.ap()[:, 8 * ch:8 * (ch + 1),
                                                       TOK * no:TOK * (no + 1)])
                            for fi in range(8):
                                f = 8 * ch + fi
                                for tt in range(4):
                                    nc.tensor.matmul(
                                        pss[tt],
                                        h1g[:, f, 128 * tt:128 * (tt + 1)],
                                        wt[:, fi, :],
                                        start=(f == 0), stop=(f == 31))
                        for tt in range(4):
                            o = o_p.tile([128, TOK], f32, tag="o")
                            nc.vector.tensor_tensor(
                                o[:], pss[tt],
                                x_sb[:, tt, TOK * no:TOK * (no + 1)], ALU.add)
                            nc.sync.dma_start(
                                out_view[:, tt, TOK * no:TOK * (no + 1)], o[:])

    nc.compile()
    return nc


def _host_prepare(x, ln1_w, ln2_w, w_qkv, w_out, w_fc1, w_fc2):
    """Fold LN weights into the following matmuls, cast to bf16, and
    rearrange weights into the layouts the kernel DMAs expect."""
    import ml_dtypes
    bf16 = ml_dtypes.bfloat16
    f8 = ml_dtypes.float8_e4m3

    x = np.asarray(x, np.float32)
    wqkv_f = (np.asarray(ln1_w, np.float32)[:, None]
              * np.asarray(w_qkv, np.float32)) * WSCALE
    wfc1_f = (np.asarray(ln2_w, np.float32)[:, None]
              * np.asarray(w_fc1, np.float32)) * WSCALE

    # wqk[p, ot, hc, o]: ot 0-7 = q col blocks, 8-15 = k col blocks
    wqk = np.ascontiguousarray(
        wqkv_f[:, :2048].reshape(8, 128, 16, 128).transpose(1, 2, 0, 3)
    ).astype(f8)
    # wv[p, hc, c] = wqkv_f[128*hc+p, 2048+c]
    wv = np.ascontiguousarray(
        wqkv_f[:, 2048:].reshape(8, 128, 1024).transpose(1, 0, 2)
    ).astype(f8)
    # wout[64*hp+d, hh, no, o] = (w_out*WS)[64*(2*hh+hp)+d, 512*no+o]
    wout = np.ascontiguousarray(
        (np.asarray(w_out, np.float32) * WSCALE).reshape(8, 2, 64, 2, TOK)
        .transpose(1, 2, 0, 3, 4).reshape(128, 8, 2, TOK)
    ).astype(f8)
    # wfc1[p, f, hc, o] = wfc1_f[128*hc+p, 128*f+o]
    wfc1 = np.ascontiguousarray(
        wfc1_f.reshape(8, 128, 32, 128).transpose(1, 2, 0, 3)
    ).astype(f8)
    # wfc2[p, f, c] = w_fc2[128*f+p, c]
    wfc2 = np.ascontiguousarray(
        np.asarray(w_fc2, np.float32).reshape(32, 128, 1024)
        .transpose(1, 0, 2)
    ).astype(bf16)

    # masks: mk[p, r, hp, qq] = 1 if 4p + r <= 4qq + k  (k = core rank)
    p_i = np.arange(128)[:, None, None, None]
    r_i = np.arange(4)[None, :, None, None]
    q_i = np.arange(128)[None, None, None, :]
    masks = [
        np.ascontiguousarray(np.broadcast_to(
            (4 * p_i + r_i <= 4 * q_i + k), (128, 4, 2, 128)).astype(bf16))
        for k in range(4)
    ]

    in_maps = []
    for i in range(NCORES):
        b, k = i // 4, i % 4
        in_maps.append({
            "xc": np.ascontiguousarray(x[b, k::4, :]),
            "wqk": wqk, "wv": wv, "wout": wout,
            "wfc1": wfc1, "wfc2": wfc2,
            "mk": masks[k],
        })
    return in_maps


def kernel(x, ln1_w, ln2_w, w_qkv, w_out, w_fc1, w_fc2):
    global _PROG
    from concourse.bass_utils import run_bass_kernel_spmd

    if _PROG is None:
        _PROG = _build_program()
    nc = _PROG

    in_maps = _host_prepare(x, ln1_w, ln2_w, w_qkv, w_out, w_fc1, w_fc2)
    res = run_bass_kernel_spmd(nc, in_maps, core_ids=list(range(NCORES)))
    out = np.empty((B, S, H), np.float32)
    for i in range(NCORES):
        b, k = i // 4, i % 4
        out[b, k::4, :] = res.results[i]["out"]
    return out
